# revision 1
# baseline (speedup 1.0000x reference)
"""TRN2 Bass kernel for nn_DS_Block (sparse attention block).

Pipeline per sample (b=32 sharded 4-per-core over 8 cores):
  x [128, 4096] --1x1conv+3tap-dwconv (folded: 3 shifted fp32r matmuls)-->
  v [128(o), 4096]           (natural layout, for a@v)
  qkT chunks [128(n), 256(o)] (transposed layout, for attention)
  G = qkT^T qkT gram accumulation -> attn blocks + q/k norms (diagonals)
  rank-based 4-way top-k masked softmax on 32x32 blocks (DVE/ACT)
  out = (proj_w @ A_blkdiag) @ v + proj_b   (P^T fused, fp32r matmuls)
"""
import os
import sys
import json
from contextlib import ExitStack

sys.path.insert(0, "/opt/trn_rl_repo")

import numpy as np
import concourse.bass as bass
import concourse.mybir as mybir
import concourse.tile as tile
from concourse.bass_utils import run_bass_kernel_spmd

F32 = mybir.dt.float32
F32R = mybir.dt.float32r
AF = mybir.ActivationFunctionType
OP = mybir.AluOpType

B, CDIM, N = 32, 128, 4096
HEADS, C = 4, 32
NCORES = 8
NSAMP = B // NCORES          # 4 samples per core
NT = N // 512                # 8 n-tiles of 512
NCH = N // 128               # 32 chunks of 128
TOPKS = [C // 2, (2 * C) // 3, (3 * C) // 4, (4 * C) // 5]  # 16,21,24,25


def _fix_sync_waits(bir: dict, max_waits: int = 1) -> dict:
    """This walrus build rejects >1 sem-wait per instruction; split the
    excess into standalone EventSemaphore waits on the same engine queue."""
    for f in bir.get("functions", []):
        for blk in f.get("blocks", []):
            out = []
            for inst in blk.get("instructions", []):
                si = inst.get("sync_info") or {}
                waits = si.get("on_wait") or []
                if len(waits) > max_waits:
                    extra, keep = waits[:-max_waits], waits[-max_waits:]
                    for j, w in enumerate(extra):
                        out.append({
                            "debug": inst.get("debug", 0),
                            "engine": inst["engine"],
                            "ins": [], "outs": [],
                            "name": f"{inst['name']}-xw{j}",
                            "opcode": "EventSemaphore",
                            "sync_info": {"on_update": [], "on_wait": [w]},
                        })
                    si["on_wait"] = keep
                    inst["sync_info"] = si
                out.append(inst)
            blk["instructions"] = out
    return bir


def _patch_nc(nc):
    orig = nc.to_json_bytes

    def fixed():
        return json.dumps(_fix_sync_waits(json.loads(orig()))).encode()

    nc.to_json_bytes = fixed
    return nc


def build_program(debug: bool = False):
    nc = bass.Bass("TRN2", target_bir_lowering=False, num_devices=NCORES)

    x_d = nc.dram_tensor("x", [NSAMP, CDIM, N], F32R, kind="ExternalInput")
    out_d = nc.dram_tensor("out", [NSAMP, CDIM, N], F32, kind="ExternalOutput")
    w3qk_d = nc.dram_tensor("w3qk", [3, CDIM, 256], F32R, kind="ExternalInput")
    w3v_d = nc.dram_tensor("w3v", [3, CDIM, CDIM], F32R, kind="ExternalInput")
    bqk2_d = nc.dram_tensor("bqk2", [CDIM, 512], F32, kind="ExternalInput")
    eqkL_d = nc.dram_tensor("eqkL", [128, 256], F32, kind="ExternalInput")
    eqkR_d = nc.dram_tensor("eqkR", [128, 256], F32, kind="ExternalInput")
    bv_d = nc.dram_tensor("bv", [CDIM, 1], F32, kind="ExternalInput")
    evL_d = nc.dram_tensor("evL", [CDIM, 1], F32, kind="ExternalInput")
    evR_d = nc.dram_tensor("evR", [CDIM, 1], F32, kind="ExternalInput")
    pwT_d = nc.dram_tensor("pwT", [CDIM, CDIM], F32, kind="ExternalInput")
    pb_d = nc.dram_tensor("pb", [CDIM, 1], F32, kind="ExternalInput")
    id128_d = nc.dram_tensor("id128", [CDIM, CDIM], F32, kind="ExternalInput")
    ones32_d = nc.dram_tensor("ones32", [1, 32], F32, kind="ExternalInput")
    lntemp_d = nc.dram_tensor("lntemp", [CDIM, 1], F32, kind="ExternalInput")
    wS_d = nc.dram_tensor("wS", [CDIM, 4], F32, kind="ExternalInput")
    dbg = {}
    if debug:
        for nm, shp in [("d_v", [CDIM, N]), ("d_qkt", [CDIM, 512]),
                        ("d_attn", [CDIM, 32]), ("d_rank", [CDIM, 32]),
                        ("d_A", [CDIM, 32]), ("d_e", [CDIM, 32]),
                        ("d_rq", [CDIM, 1]), ("d_rk", [CDIM, 1]),
                        ("d_RK", [CDIM, 32]), ("d_S", [CDIM, 4])]:
            dbg[nm] = nc.dram_tensor(nm, shp, F32, kind="ExternalOutput")

    with tile.TileContext(nc) as tc, ExitStack() as es:
        wp = es.enter_context(tc.tile_pool(name="wp", bufs=1))
        xp = es.enter_context(tc.tile_pool(name="xp", bufs=3))
        vp = es.enter_context(tc.tile_pool(name="vp", bufs=2))
        qkp = es.enter_context(tc.tile_pool(name="qkp", bufs=6))
        ap = es.enter_context(tc.tile_pool(name="ap", bufs=2))
        op_ = es.enter_context(tc.tile_pool(name="op", bufs=2))
        ps_v = es.enter_context(tc.tile_pool(name="ps_v", bufs=2, space="PSUM"))
        ps_qk = es.enter_context(tc.tile_pool(name="ps_qk", bufs=3, space="PSUM"))
        ps_g = es.enter_context(tc.tile_pool(name="ps_g", bufs=2, space="PSUM"))
        ps_s = es.enter_context(tc.tile_pool(name="ps_s", bufs=1, space="PSUM"))

        # ---- constants ----
        w3qk = wp.tile([CDIM, 3, 256], F32R)
        w3v = wp.tile([CDIM, 3, CDIM], F32R)
        bqk2 = wp.tile([CDIM, 512], F32)
        eqkL = wp.tile([128, 256], F32)
        eqkR = wp.tile([128, 256], F32)
        bv = wp.tile([CDIM, 1], F32)
        evL = wp.tile([CDIM, 1], F32)
        evR = wp.tile([CDIM, 1], F32)
        pwT = wp.tile([CDIM, CDIM], F32)
        pb = wp.tile([CDIM, 1], F32)
        id128 = wp.tile([CDIM, CDIM], F32)
        ones32 = wp.tile([1, 32], F32)
        lntemp = wp.tile([CDIM, 1], F32)
        wS = wp.tile([CDIM, 4], F32)
        for t in range(3):
            nc.sync.dma_start(w3qk[:, t, :], w3qk_d[t])
            nc.sync.dma_start(w3v[:, t, :], w3v_d[t])
        for t_, d_ in [(bqk2, bqk2_d),
                       (eqkL, eqkL_d), (eqkR, eqkR_d), (bv, bv_d),
                       (evL, evL_d), (evR, evR_d), (pwT, pwT_d), (pb, pb_d),
                       (id128, id128_d), (ones32, ones32_d),
                       (lntemp, lntemp_d), (wS, wS_d)]:
            nc.sync.dma_start(t_[:], d_[:])

        for s in range(NSAMP):
            # ---- load x with 1-col zero halo on each side ----
            xt = xp.tile([CDIM, N + 2], F32R, tag="x")
            if s < 3:  # halo cols persist across slot reuse (bufs=3)
                xf = xt[:].bitcast(F32)
                nc.vector.memset(xf[:, 0:1], 0.0)
                nc.vector.memset(xf[:, N + 1:N + 2], 0.0)
            for q in range(8):
                nc.sync.dma_start(xt[:, 1 + q * 512:1 + (q + 1) * 512],
                                  x_d[s, :, q * 512:(q + 1) * 512])

            # ---- qkT-pass + gram accumulation ----
            G01 = ps_g.tile([CDIM, 512], F32, tag="g")  # [QtQ|QtK] 0:256, [KtQ|KtK] 256:512
            for cp in range(0 if os.environ.get("ABL_NOQKT") else NCH // 2):
                qkps = ps_qk.tile([CDIM, 512], F32, tag="qk")
                for half in range(2):
                    ch = cp * 2 + half
                    for t in range(3):
                        nc.tensor.matmul(
                            qkps[:, half * 256:(half + 1) * 256],
                            xt[:, ch * 128 + t: ch * 128 + t + 128],
                            w3qk[:, t, :], start=(t == 0), stop=(t == 2))
                qkt = qkp.tile([CDIM, 512], F32R, tag="qkt")
                nc.vector.scalar_tensor_tensor(qkt[:], qkps[:], 1.0, bqk2[:],
                                               OP.mult, OP.add)
                if cp == 0:
                    nc.vector.tensor_tensor(qkt[:, 0:256], qkt[:, 0:256],
                                            eqkL[:], OP.add)
                if cp == NCH // 2 - 1:
                    nc.vector.tensor_tensor(qkt[:, 256:512],
                                            qkt[:, 256:512], eqkR[:],
                                            OP.add)
                if debug and s == 0 and cp == 0:
                    nc.sync.dma_start(dbg["d_qkt"][:], qkt[:].bitcast(F32))
                for half in range(2):
                    ch = cp * 2 + half
                    off = half * 256
                    nc.tensor.matmul(G01[:, 0:256], qkt[:, off:off + 128],
                                     qkt[:, off:off + 256],
                                     start=(ch == 0), stop=(ch == NCH - 1))
                    nc.tensor.matmul(G01[:, 256:512],
                                     qkt[:, off + 128:off + 256],
                                     qkt[:, off:off + 256],
                                     start=(ch == 0), stop=(ch == NCH - 1))

            # ---- v-pass: v[o, n] = sum_t W3v_t^T.T @ x_shift ----
            vt = vp.tile([CDIM, N], F32R, tag="v")
            for nt in range(0 if os.environ.get("ABL_NOV") else NT):
                vps = ps_v.tile([CDIM, 512], F32, tag="vv")
                for t in range(3):
                    nc.tensor.matmul(vps[:], w3v[:, t, :],
                                     xt[:, nt * 512 + t: nt * 512 + t + 512],
                                     start=(t == 0), stop=(t == 2))
                nc.scalar.activation(vt[:, nt * 512:(nt + 1) * 512], vps[:],
                                     AF.Identity, bias=bv[:])
            nc.vector.tensor_tensor(vt[:, 0:1], vt[:, 0:1], evL[:], OP.add)
            nc.vector.tensor_tensor(vt[:, N - 1:N], vt[:, N - 1:N], evR[:],
                                    OP.add)
            if debug and s == 0:
                nc.sync.dma_start(dbg["d_v"][:], vt[:].bitcast(F32))

            # ---- attention phase (per sample, [128, 32] tiles) ----
            ABL_NOPHASE = bool(os.environ.get("ABL_NOPHASE"))
            sm = ps_s.tile([CDIM, 512], F32, tag="sm")  # rkT 0:128, RK 128:160, PT 256:384
            attn_raw = ap.tile([CDIM, 32], F32, tag="attn")
            for h in range(0 if ABL_NOPHASE else HEADS):
                nc.scalar.copy(attn_raw[32 * h:32 * (h + 1), :],
                               G01[32 * h:32 * (h + 1),
                                   128 + 32 * h:128 + 32 * (h + 1)])
            A = ap.tile([CDIM, 32], F32, tag="A")
            if ABL_NOPHASE:
                nc.vector.memset(A[:], 0.03)
            else:
                qsq = ap.tile([CDIM, 1], F32, tag="qsq")
                ksq = ap.tile([CDIM, 1], F32, tag="ksq")
                scr = ap.tile([CDIM, 128], F32, tag="scr")
                nc.vector.tensor_tensor(scr[:], G01[:, 384:512], id128[:], OP.mult)
                nc.vector.tensor_reduce(ksq[:], scr[:], mybir.AxisListType.X,
                                        OP.add)
                nc.vector.tensor_tensor(scr[:], G01[:, 0:128], id128[:], OP.mult)
                nc.vector.tensor_reduce(qsq[:], scr[:], mybir.AxisListType.X,
                                        OP.add)
                rq = ap.tile([CDIM, 1], F32, tag="rq")
                rk = ap.tile([CDIM, 1], F32, tag="rk")
                nc.scalar.activation(rk[:], ksq[:], AF.Ln)
                nc.scalar.activation(rk[:], rk[:], AF.Exp, scale=-0.5)
                nc.scalar.activation(rq[:], qsq[:], AF.Ln)
                nc.scalar.activation(rq[:], rq[:], AF.Exp, bias=lntemp[:],
                                     scale=-0.5)
                # broadcast rk over its head-block columns: transpose + 4 K=1 mms
                nc.tensor.transpose(sm[0:1, 0:128], rk[:], id128[:])
                rkrow = ap.tile([1, 128], F32, tag="rkrow")
                nc.scalar.copy(rkrow[:], sm[0:1, 0:128])
                for h in range(HEADS):
                    nc.tensor.matmul(sm[32 * h:32 * (h + 1), 128:160], ones32[:],
                                     rkrow[0:1, 32 * h:32 * (h + 1)],
                                     tile_position=(0, 32 * h))
                attn_s = ap.tile([CDIM, 32], F32, tag="attn_s")
                nc.vector.scalar_tensor_tensor(attn_s[:], attn_raw[:], rq[:],
                                               sm[:, 128:160], OP.mult, OP.mult)
                if debug and s == 0:
                    nc.sync.dma_start(dbg["d_attn"][:], attn_s[:])
                    nc.sync.dma_start(dbg["d_rq"][:], rq[:])
                    nc.sync.dma_start(dbg["d_rk"][:], rk[:])
                    rkb = ap.tile([CDIM, 32], F32, tag="rkb")
                    nc.vector.tensor_copy(rkb[:], sm[:, 128:160])
                    nc.sync.dma_start(dbg["d_RK"][:], rkb[:])
                # e = exp(attn_s - rowmax)
                nmx = ap.tile([CDIM, 1], F32, tag="nmx")
                nc.vector.tensor_reduce(nmx[:], attn_s[:], mybir.AxisListType.X,
                                        OP.max, negate=True)
                e = ap.tile([CDIM, 32], F32, tag="e")
                nc.scalar.activation(e[:], attn_s[:], AF.Exp, bias=nmx[:])
                # ranks: rank[c,d] = #{d' : attn[c,d'] > attn[c,d]}
                rank = ap.tile([CDIM, 32], F32, tag="rank")
                cmp = ap.tile([CDIM, C, C], F32, tag="cmp")
                nc.vector.tensor_tensor(
                    cmp[:], attn_s[:, :, None].to_broadcast((CDIM, C, C)),
                    attn_s[:, None, :].to_broadcast((CDIM, C, C)), OP.is_lt)
                nc.vector.tensor_reduce(rank[:], cmp[:],
                                        mybir.AxisListType.X, OP.add)
                # masked sums S_i and masked-e tiles
                S = ap.tile([CDIM, 4], F32, tag="S")
                me = [ap.tile([CDIM, 32], F32, tag=f"me{i}", name=f"me{i}") for i in range(4)]
                for i, kk in enumerate(TOPKS):
                    nc.vector.scalar_tensor_tensor(me[i][:], rank[:], float(kk),
                                                   e[:], OP.is_lt, OP.mult,
                                                   accum_out=S[:, i:i + 1])
                R = ap.tile([CDIM, 4], F32, tag="R")
                nc.vector.reciprocal(R[:], S[:])
                nc.vector.tensor_tensor(R[:], R[:], wS[:], OP.mult)
                A = ap.tile([CDIM, 32], F32, tag="A")
                nc.vector.tensor_scalar_mul(A[:], me[0][:], R[:, 0:1])
                for i in range(1, 4):
                    nc.vector.scalar_tensor_tensor(A[:], me[i][:], R[:, i:i + 1],
                                                   A[:], OP.mult, OP.add)
                if debug and s == 0:
                    nc.sync.dma_start(dbg["d_rank"][:], rank[:])
                    nc.sync.dma_start(dbg["d_A"][:], A[:])
                    nc.sync.dma_start(dbg["d_e"][:], e[:])
                    nc.sync.dma_start(dbg["d_S"][:], S[:])
            # P^T = A_blkdiag^T @ proj_w^T  (lhsT = block-diag of A)
            BD = ap.tile([CDIM, CDIM], F32, tag="BD")
            if s < 2:  # off-diag zeros persist across slot reuse (bufs=2)
                nc.vector.memset(BD[:], 0.0)
            for h in range(HEADS):
                nc.scalar.copy(BD[32 * h:32 * (h + 1),
                                  32 * h:32 * (h + 1)],
                               A[32 * h:32 * (h + 1), :])
            nc.tensor.matmul(sm[:, 256:384], BD[:], pwT[:])
            PT = ap.tile([CDIM, CDIM], F32R, tag="PT")
            nc.scalar.copy(PT[:], sm[:, 256:384])
            # av + out
            ot = op_.tile([CDIM, N], F32, tag="out")
            for nt in range(0 if os.environ.get("ABL_NOAV") else NT):
                avps = ps_v.tile([CDIM, 512], F32, tag="vv")
                nc.tensor.matmul(avps[:], PT[:],
                                 vt[:, nt * 512:(nt + 1) * 512])
                nc.scalar.activation(ot[:, nt * 512:(nt + 1) * 512], avps[:],
                                     AF.Identity, bias=pb[:])
            for q in range(8):
                nc.sync.dma_start(out_d[s, :, q * 512:(q + 1) * 512],
                                  ot[:, q * 512:(q + 1) * 512])

    _patch_nc(nc)
    return nc


_NC_CACHE = {}


def _get_nc(debug=False):
    key = bool(debug)
    if key not in _NC_CACHE:
        _NC_CACHE[key] = build_program(debug=key)
    return _NC_CACHE[key]


def make_inputs(x, qkv_w, qkv_b, dw_w, dw_b, proj_w, proj_b, temperature,
                attn_w):
    """Host-side weight prep -> per-core input maps."""
    x = np.ascontiguousarray(np.asarray(x, np.float32)[:, :, :, 0])
    qkv_w = np.asarray(qkv_w, np.float32)
    qkv_b = np.asarray(qkv_b, np.float32)
    dw_w = np.asarray(dw_w, np.float32)
    dw_b = np.asarray(dw_b, np.float32)
    proj_w = np.asarray(proj_w, np.float32)
    proj_b = np.asarray(proj_b, np.float32)
    temperature = np.asarray(temperature, np.float32).reshape(HEADS)
    attn_w = np.asarray(attn_w, np.float32)

    dwk = dw_w[:, 0, :, 1]                       # [384, 3]
    w3qk = np.stack([(qkv_w[:256] * dwk[:256, t:t + 1]).T.copy()
                     for t in range(3)])          # [3, 128, 256]
    w3v = np.stack([(qkv_w[256:] * dwk[256:, t:t + 1]).T.copy()
                    for t in range(3)])           # [3, 128, 128]
    bqk = qkv_b[:256] * dwk[:256].sum(1) + dw_b[:256]          # [256]
    bqk2 = np.tile(np.concatenate([bqk, bqk])[None, :], (CDIM, 1))  # [128,512]
    eqkL = np.zeros((128, 256), np.float32)
    eqkL[0] = -qkv_b[:256] * dwk[:256, 0]
    eqkR = np.zeros((128, 256), np.float32)
    eqkR[127] = -qkv_b[:256] * dwk[:256, 2]
    bv = (qkv_b[256:] * dwk[256:].sum(1) + dw_b[256:])[:, None]  # [128,1]
    evL = (-qkv_b[256:] * dwk[256:, 0])[:, None]
    evR = (-qkv_b[256:] * dwk[256:, 2])[:, None]
    pwT = proj_w.T.copy()                        # [c, o]
    pb = proj_b[:, None].copy()
    id128 = np.eye(CDIM, dtype=np.float32)
    ones32 = np.ones((1, 32), np.float32)
    lntemp = np.repeat(np.log(np.maximum(temperature, 1e-30)), C)[:, None]
    lntemp = np.ascontiguousarray(lntemp, np.float32)
    wS = np.tile(attn_w[None, :], (CDIM, 1))

    const = dict(w3qk=w3qk, w3v=w3v, bqk2=bqk2.astype(np.float32),
                 eqkL=eqkL.astype(np.float32), eqkR=eqkR.astype(np.float32),
                 bv=bv.astype(np.float32), evL=evL.astype(np.float32),
                 evR=evR.astype(np.float32), pwT=pwT, pb=pb, id128=id128,
                 ones32=ones32, lntemp=lntemp, wS=wS.astype(np.float32))
    maps = []
    for i in range(NCORES):
        m = dict(const)
        m["x"] = np.ascontiguousarray(x[i * NSAMP:(i + 1) * NSAMP])
        maps.append(m)
    return maps


def kernel(**inputs):
    nc = _get_nc(debug=False)
    maps = make_inputs(**inputs)
    res = run_bass_kernel_spmd(nc, maps, list(range(NCORES)))
    outs = [res.results[i]["out"] for i in range(NCORES)]
    full = np.concatenate(outs, axis=0)          # [32, 128, 4096]
    return full[:, :, :, None].astype(np.float32)



# revision 12
# speedup vs baseline: 1.2261x; 1.2261x over previous
"""TRN2 Bass kernel for nn_DS_Block (sparse attention block).

Per sample (b=32, 4 per core over 8 cores), all-fused pipeline:
  x --fp8 DoubleRow 3-tap conv--> qkT [n,256] (fp8, scale 2^4)
  G = qkT^T qkT (fp8 DoubleRow gram) + rank-1 bf16 bias corrections
  rank-based 4-way top-k masked softmax on 32x32 head blocks
  P = proj @ A_blkdiag folded into the v-conv: out = sum_t (P Wv_t) x_shift
  (fp32r out conv; biases via PT^T @ [bv|evL|evR] matmul)
"""
import os
import sys
import json
from contextlib import ExitStack

sys.path.insert(0, "/opt/trn_rl_repo")

import numpy as np
import ml_dtypes
import bass_rust
import concourse.bass as bass
import concourse.mybir as mybir
import concourse.tile as tile
from concourse.bass_utils import run_bass_kernel_spmd

F32 = mybir.dt.float32
F32R = mybir.dt.float32r
F8 = mybir.dt.float8e4
BF16 = mybir.dt.bfloat16
AF = mybir.ActivationFunctionType
OP = mybir.AluOpType
DR = mybir.MatmulPerfMode.DoubleRow

B, CDIM, N = 32, 128, 4096
HEADS, C = 4, 32
NCORES = 8
NSAMP = B // NCORES
NPAIR = N // 256            # 16 chunk-pairs of 2x128
TOPKS = [C // 2, (2 * C) // 3, (3 * C) // 4, (4 * C) // 5]  # 16,21,24,25
SQK = 2.0 ** 6              # w8qk host scale
SQ8 = 2.0 ** 4              # qk8 fp8 scale
W2 = 512                    # per-chunk dual-plane block (planes at +0 / +256)
NCH = N // 128
HALF_N = (N // 2) * 1.0

# wf8 layout: [128, 4*256 + 130]: w8qk taps at j*256, ones-pair at 1024 & 1152
WF8_COLS = 1154
# wb16 layout: [128, 771]: w3vn (3*128), pwT (128), bv3 (3), b16row in row0 at 515:771
WB16_COLS = 771
# wf32 layout: [128, 391]: id128 (128), pb (1), lntemp (1), wS (4), onescol (1),
# eqkR-block (256: rows 96:127 zero, row 127 = eqkR)
WF32_COLS = 391
# wrow layout: [1, 800] f32: halfNb 0:256, eqkL 256:512, eqkR 512:768, ones32 768:800
WROW_COLS = 800


def _fix_sync_waits(bir: dict, max_waits: int = 1) -> dict:
    """This walrus build rejects >1 sem-wait per instruction; split the
    excess into standalone EventSemaphore waits on the same engine queue."""
    for f in bir.get("functions", []):
        for blk in f.get("blocks", []):
            out = []
            for inst in blk.get("instructions", []):
                si = inst.get("sync_info") or {}
                waits = si.get("on_wait") or []
                if len(waits) > max_waits:
                    extra, keep = waits[:-max_waits], waits[-max_waits:]
                    for j, w in enumerate(extra):
                        out.append({
                            "debug": inst.get("debug", 0),
                            "engine": inst["engine"],
                            "ins": [], "outs": [],
                            "name": f"{inst['name']}-xw{j}",
                            "opcode": "EventSemaphore",
                            "sync_info": {"on_update": [], "on_wait": [w]},
                        })
                    si["on_wait"] = keep
                    inst["sync_info"] = si
                out.append(inst)
            blk["instructions"] = out
    return bir


def _patch_nc(nc):
    orig = nc.to_json_bytes

    def fixed():
        return json.dumps(_fix_sync_waits(json.loads(orig()))).encode()

    nc.to_json_bytes = fixed
    return nc


def _ap(base, offset, dims):
    c = base.copy()
    c.ap = bass_rust.VecI64Pair(dims)
    c.offset = offset
    return c


def build_program():
    nc = bass.Bass("TRN2", target_bir_lowering=False, num_devices=NCORES)

    x_d = nc.dram_tensor("x", [NSAMP, CDIM, N], F32R, kind="ExternalInput")
    out_d = nc.dram_tensor("out", [NSAMP, CDIM, N], F32, kind="ExternalOutput")
    wf8_d = nc.dram_tensor("wf8", [CDIM, WF8_COLS], F8, kind="ExternalInput")
    wb16_d = nc.dram_tensor("wb16", [CDIM, WB16_COLS], BF16, kind="ExternalInput")
    wf32_d = nc.dram_tensor("wf32", [CDIM, WF32_COLS], F32, kind="ExternalInput")
    wrow_d = nc.dram_tensor("wrow", [1, WROW_COLS], F32, kind="ExternalInput")

    with tile.TileContext(nc) as tc, ExitStack() as es:
        wp = es.enter_context(tc.tile_pool(name="wp", bufs=1))
        xp = es.enter_context(tc.tile_pool(name="xp", bufs=3))
        x8p = es.enter_context(tc.tile_pool(name="x8p", bufs=2))
        qp = es.enter_context(tc.tile_pool(name="qp", bufs=3))
        bp = es.enter_context(tc.tile_pool(name="bp", bufs=2))
        op_ = es.enter_context(tc.tile_pool(name="op", bufs=2))
        ps_qk = es.enter_context(tc.tile_pool(name="ps_qk", bufs=2, space="PSUM"))
        ps_g = es.enter_context(tc.tile_pool(name="ps_g", bufs=2, space="PSUM"))
        ps_av = es.enter_context(tc.tile_pool(name="ps_av", bufs=2, space="PSUM"))
        ps_sm = es.enter_context(tc.tile_pool(name="ps_sm", bufs=2, space="PSUM"))

        # ---- constants ----
        wf8 = wp.tile([CDIM, WF8_COLS], F8)
        wb16 = wp.tile([CDIM, WB16_COLS], BF16)
        wf32 = wp.tile([CDIM, WF32_COLS], F32)
        wrow = wp.tile([1, WROW_COLS], F32)
        nc.sync.dma_start(wf8[:], wf8_d[:])
        nc.sync.dma_start(wb16[:], wb16_d[:])
        nc.sync.dma_start(wf32[:], wf32_d[:])
        nc.sync.dma_start(wrow[:], wrow_d[:])
        id128 = wf32[:, 0:128]
        pb = wf32[:, 128:129]
        lntemp = wf32[:, 129:130]
        wS = wf32[:, 130:134]
        onescol = wf32[:, 134:135]
        eqkRblk = wf32[96:128, 135:391]
        w3vn = lambda t: wb16[:, t * 128:(t + 1) * 128]
        pwTb = wb16[:, 384:512]
        bv3 = wb16[:, 512:515]
        b16row = wb16[0:1, 515:771]       # scaled bias row (2^4), bf16
        halfNb = wrow[0:1, 0:256]
        eqkL = wrow[0:1, 256:512]
        eqkR = wrow[0:1, 512:768]
        ones32 = wrow[0:1, 768:800]

        PW = list(wf8[:].ap[0])           # wf8 partition AP pair

        # per-sample emission state
        xts, x8s, sms, Gs, Bq = {}, {}, {}, {}, {}
        tiles = {}

        def emit_dma_x(s):
            xt = xp.tile([CDIM, N + 8], F32R, tag="x", name=f"x{s}")
            xts[s] = xt
            if s < 3:
                xf = xt[:].bitcast(F32)
                nc.vector.memset(xf[:, 0:1], 0.0)
                nc.vector.memset(xf[:, N + 1:N + 8], 0.0)
            for h in range(2):
                nc.sync.dma_start(xt[:, 1 + h * 2048:1 + (h + 1) * 2048],
                                  x_d[s, :, h * 2048:(h + 1) * 2048])

        def emit_convert(s):
            # fp8 per-chunk dual-plane blocks: chunk ch at col ch*W2:
            #   [0:132]   = xpad[ch*128 + 0 .. 131]  (tap base, shift 0)
            #   [256:388] = xpad[ch*128 + 1 .. 132]  (shift +1)
            xt = xts[s]
            x8 = x8p.tile([CDIM, NCH * W2], F8, tag="x8", name=f"x8_{s}")
            x8s[s] = x8
            xb, o8 = xt[:], x8[:]
            PXT, PX8 = list(xb.ap[0]), list(o8.ap[0])
            # plane0 halves on ACT, plane1 halves on Pool
            for h in range(2):
                nc.scalar.activation(
                    _ap(o8, h * 16 * W2, [PX8, [W2, 16], [1, 132]]),
                    _ap(xb, h * 16 * 128, [PXT, [128, 16], [1, 132]]),
                    AF.Identity)
            for h in range(2):
                nc.gpsimd.tensor_copy(
                    _ap(o8, h * 16 * W2 + 256, [PX8, [W2, 16], [1, 132]]),
                    _ap(xb, h * 16 * 128 + 1, [PXT, [128, 16], [1, 132]]))

        def emit_A(s, interleave=None):
            # qkT + gram + srow for sample s; pops B(s-1) closures as it goes
            x8 = x8s[s]
            G = ps_g.tile([CDIM, 512], F32, tag="g", name=f"G{s}")
            Gs[s] = G
            sm = ps_sm.tile([CDIM, 512], F32, tag="sm", name=f"sm{s}")
            sms[s] = sm
            x8b = x8[:]
            PX = list(x8b.ap[0])
            for cp in range(NPAIR):
                qkps = ps_qk.tile([CDIM, 512], F32, tag="qk", name=f"qkps{s}_{cp}")
                for half in range(2):
                    ch = cp * 2 + half
                    o = half * 256
                    # taps (0,1): lhsT = chunk dual-plane block, j stride 132
                    nc.tensor.matmul(
                        qkps[:, o:o + 256],
                        _ap(x8b, ch * W2, [PX, [256, 2], [1, 128]]),
                        _ap(wf8[:], 0, [PW, [256, 2], [1, 256]]),
                        perf_mode=DR, start=True, stop=False)
                    # taps (2,3): block offsets +2; tap3 weights are zero
                    nc.tensor.matmul(
                        qkps[:, o:o + 256],
                        _ap(x8b, ch * W2 + 2, [PX, [256, 2], [1, 128]]),
                        _ap(wf8[:], 512, [PW, [256, 2], [1, 256]]),
                        perf_mode=DR, start=False, stop=True)
                qk8 = qp.tile([CDIM, 512], F8, tag="qk8", name=f"qk8_{s}_{cp}")
                eng = nc.vector if cp % 2 == 0 else nc.scalar
                if cp % 2 == 0:
                    nc.vector.tensor_scalar_mul(qk8[:], qkps[:], SQ8 / SQK)
                else:
                    nc.scalar.activation(qk8[:], qkps[:], AF.Identity,
                                         scale=SQ8 / SQK)
                if cp == 0:   # n=0 edge fix (bias tap overcount)
                    nc.vector.tensor_tensor(qk8[0:1, 0:256], qk8[0:1, 0:256],
                                            eqkL[:], OP.add)
                if cp == NPAIR - 1:  # n=4095 edge fix (rows 96:127 add zero)
                    nc.vector.tensor_tensor(qk8[96:128, 256:512],
                                            qk8[96:128, 256:512], eqkRblk,
                                            OP.add)
                qb = qk8[:]
                PQ = list(qb.ap[0])
                st, sp_ = (cp == 0), False
                # gram A: [QtQ | QtK] <- q^T (q|k), both chunks via DoubleRow
                nc.tensor.matmul(G[:, 0:256],
                                 _ap(qb, 0, [PQ, [256, 2], [1, 128]]),
                                 _ap(qb, 0, [PQ, [256, 2], [1, 256]]),
                                 perf_mode=DR, start=st, stop=sp_)
                # gram B: KtK at cols 256:384
                nc.tensor.matmul(G[:, 256:384],
                                 _ap(qb, 128, [PQ, [256, 2], [1, 128]]),
                                 _ap(qb, 128, [PQ, [256, 2], [1, 128]]),
                                 perf_mode=DR, start=st, stop=sp_)
                # srow: column sums via ones-pair
                nc.tensor.matmul(sm[0:1, 0:256],
                                 _ap(wf8[:], 1024, [PW, [128, 2], [1, 1]]),
                                 _ap(qb, 0, [PQ, [256, 2], [1, 256]]),
                                 perf_mode=DR, start=st, stop=(cp == NPAIR - 1))
                if interleave:
                    interleave(cp)
            # bias as rank-1: G += b (x) u + u (x) b with u = srow + (N/2) b
            u16 = bp.tile([1, 256], BF16, tag="u16", name=f"u16_{s}")
            nc.vector.scalar_tensor_tensor(u16[:], sm[0:1, 0:256], 1.0,
                                           halfNb[:], OP.mult, OP.add)
            ub = u16[:]
            nc.tensor.matmul(G[:, 0:256], b16row[0:1, 0:128], ub,
                             start=False, stop=False)
            nc.tensor.matmul(G[:, 0:256], u16[0:1, 0:128], b16row[:],
                             start=False, stop=True)
            nc.tensor.matmul(G[:, 256:384], b16row[0:1, 128:256],
                             u16[0:1, 128:256], start=False, stop=False)
            nc.tensor.matmul(G[:, 256:384], u16[0:1, 128:256],
                             b16row[0:1, 128:256], start=False, stop=True)

        def build_B(s):
            # attention phase closures (interleaved into A(s+1) emission)
            G, sm = Gs[s], sms[s]
            t = {}
            t["scrQ"] = bp.tile([CDIM, 128], F32, tag="scrQ", name=f"scrQ{s}")
            t["scrK"] = bp.tile([CDIM, 128], F32, tag="scrK", name=f"scrK{s}")
            t["qsq"] = bp.tile([CDIM, 1], F32, tag="qsq", name=f"qsq{s}")
            t["rq"] = bp.tile([CDIM, 1], F32, tag="rq", name=f"rq{s}")
            t["rkrow"] = bp.tile([1, 128], F32, tag="rkrow", name=f"rkrow{s}")
            t["attn_raw"] = bp.tile([CDIM, 32], F32, tag="attn_raw", name=f"ar{s}")
            t["attn_s"] = bp.tile([CDIM, 32], F32, tag="attn_s", name=f"as{s}")
            t["nmx"] = bp.tile([CDIM, 1], F32, tag="nmx", name=f"nmx{s}")
            t["e"] = bp.tile([CDIM, 32], F32, tag="e", name=f"e{s}")
            t["cmp"] = bp.tile([CDIM, C, C], F32, tag="cmp", name=f"cmp{s}")
            t["rank"] = bp.tile([CDIM, 32], F32, tag="rank", name=f"rank{s}")
            t["S"] = bp.tile([CDIM, 4], F32, tag="S", name=f"S{s}")
            t["R"] = bp.tile([CDIM, 4], F32, tag="R", name=f"R{s}")
            t["A"] = bp.tile([CDIM, 32], F32, tag="A", name=f"A{s}")
            t["me"] = [bp.tile([CDIM, 32], F32, tag=f"me{i}", name=f"me{i}_{s}")
                       for i in range(4)]
            tiles[s] = t
            ops = []
            ops.append(lambda: nc.vector.tensor_tensor(
                t["scrK"][:], G[:, 256:384], id128, OP.mult))
            ops.append(lambda: nc.tensor.matmul(
                sm[0:1, 256:384], onescol, t["scrK"][:]))
            ops.append(lambda: (
                nc.scalar.activation(t["rkrow"][:], sm[0:1, 256:384], AF.Ln),
                nc.scalar.activation(t["rkrow"][:], t["rkrow"][:], AF.Exp,
                                     scale=-0.5)))
            ops.append(lambda: nc.vector.tensor_tensor(
                t["scrQ"][:], G[:, 0:128], id128, OP.mult))
            ops.append(lambda: nc.vector.tensor_reduce(
                t["qsq"][:], t["scrQ"][:], mybir.AxisListType.X, OP.add))
            ops.append(lambda: (
                nc.scalar.activation(t["rq"][:], t["qsq"][:], AF.Ln),
                nc.scalar.activation(t["rq"][:], t["rq"][:], AF.Exp,
                                     bias=lntemp, scale=-0.5)))
            def rk_mms():
                for h in range(HEADS):
                    nc.tensor.matmul(sm[32 * h:32 * (h + 1), 384:416],
                                     ones32[:], t["rkrow"][0:1, 32 * h:32 * h + 32],
                                     tile_position=(0, 32 * h))
            ops.append(rk_mms)
            def ar_copies():
                for h in range(HEADS):
                    nc.scalar.copy(t["attn_raw"][32 * h:32 * (h + 1), :],
                                   G[32 * h:32 * (h + 1),
                                     128 + 32 * h:128 + 32 * (h + 1)])
            ops.append(ar_copies)
            ops.append(lambda: nc.vector.scalar_tensor_tensor(
                t["attn_s"][:], t["attn_raw"][:], t["rq"][:],
                sm[:, 384:416], OP.mult, OP.mult))
            ops.append(lambda: nc.vector.tensor_reduce(
                t["nmx"][:], t["attn_s"][:], mybir.AxisListType.X, OP.max,
                negate=True))
            ops.append(lambda: nc.scalar.activation(
                t["e"][:], t["attn_s"][:], AF.Exp, bias=t["nmx"][:]))
            ops.append(lambda: nc.vector.tensor_tensor(
                t["cmp"][:], t["attn_s"][:, :, None].to_broadcast((CDIM, C, C)),
                t["attn_s"][:, None, :].to_broadcast((CDIM, C, C)), OP.is_lt))
            ops.append(lambda: nc.vector.tensor_reduce(
                t["rank"][:], t["cmp"][:], mybir.AxisListType.X, OP.add))
            def masked(i):
                return lambda: nc.vector.scalar_tensor_tensor(
                    t["me"][i][:], t["rank"][:], float(TOPKS[i]), t["e"][:],
                    OP.is_lt, OP.mult, accum_out=t["S"][:, i:i + 1])
            for i in range(4):
                ops.append(masked(i))
            def combine():
                nc.vector.reciprocal(t["R"][:], t["S"][:])
                nc.vector.tensor_tensor(t["R"][:], t["R"][:], wS, OP.mult)
                nc.vector.tensor_scalar_mul(t["A"][:], t["me"][0][:],
                                            t["R"][:, 0:1])
                for i in range(1, 4):
                    nc.vector.scalar_tensor_tensor(
                        t["A"][:], t["me"][i][:], t["R"][:, i:i + 1], t["A"][:],
                        OP.mult, OP.add)
            ops.append(combine)
            Bq[s] = ops

        def emit_C(s):
            # P-fold + out conv + store for sample s (after B(s) complete)
            t, sm, xt = tiles[s], sms[s], xts[s]
            BD = bp.tile([CDIM, CDIM], BF16, tag="BD", name=f"BD{s}")
            if s < 2:
                nc.vector.memset(BD[:], 0.0)
            for h in range(HEADS):
                nc.scalar.copy(BD[32 * h:32 * (h + 1), 32 * h:32 * (h + 1)],
                               t["A"][32 * h:32 * (h + 1), :])
            nc.tensor.matmul(sm[:, 384:512], BD[:], pwTb)
            PTb = bp.tile([CDIM, CDIM], BF16, tag="PTb", name=f"PTb{s}")
            nc.scalar.copy(PTb[:], sm[:, 384:512])
            MTps = ps_av.tile([CDIM, 384], F32, tag="av", name=f"MTps{s}")
            for tp in range(3):
                nc.tensor.matmul(MTps[:, tp * 128:(tp + 1) * 128],
                                 w3vn(tp), PTb[:])
            MTf = bp.tile([CDIM, 384], F32R, tag="MTf", name=f"MTf{s}")
            nc.scalar.copy(MTf[:], MTps[:])
            nc.tensor.matmul(sm[:, 416:419], PTb[:], bv3)
            biasv = bp.tile([CDIM, 1], F32, tag="biasv", name=f"biasv{s}")
            nc.vector.tensor_tensor(biasv[:], pb, sm[:, 416:417], OP.add)
            ot = op_.tile([CDIM, N], F32, tag="out", name=f"ot{s}")
            for q in range(8):
                avps = ps_av.tile([CDIM, 512], F32, tag="av", name=f"av{s}_{q}")
                for tp in range(3):
                    nc.tensor.matmul(avps[:],
                                     MTf[:, tp * 128:(tp + 1) * 128],
                                     xt[:, q * 512 + tp:q * 512 + tp + 512],
                                     start=(tp == 0), stop=(tp == 2))
                nc.scalar.activation(ot[:, q * 512:(q + 1) * 512], avps[:],
                                     AF.Identity, bias=biasv[:])
                if q == 0:
                    nc.vector.tensor_tensor(ot[:, 0:1], ot[:, 0:1],
                                            sm[:, 417:418], OP.add)
                if q == 7:
                    nc.vector.tensor_tensor(ot[:, N - 1:N], ot[:, N - 1:N],
                                            sm[:, 418:419], OP.add)
                if q == 3:
                    nc.sync.dma_start(out_d[s, :, 0:2048], ot[:, 0:2048])
            nc.sync.dma_start(out_d[s, :, 2048:N], ot[:, 2048:N])

        # ---- pipeline ----
        emit_dma_x(0)
        emit_convert(0)
        emit_dma_x(1)
        for s in range(NSAMP):
            pend = list(Bq.get(s - 1, []))
            def inter(cp, _p=pend):
                for _ in range(2):
                    if _p:
                        _p.pop(0)()
            emit_A(s, interleave=inter if pend else None)
            for f in pend:
                f()
            Bq.pop(s - 1, None)
            build_B(s)
            if s >= 1:
                emit_C(s - 1)
            if s + 1 < NSAMP:
                emit_convert(s + 1)
            if s + 2 < NSAMP:
                emit_dma_x(s + 2)
        for f in Bq[NSAMP - 1]:
            f()
        emit_C(NSAMP - 1)

    _patch_nc(nc)
    return nc


_NC_CACHE = {}


def _get_nc(debug=False):
    key = bool(debug)
    if key not in _NC_CACHE:
        _NC_CACHE[key] = build_program()
    return _NC_CACHE[key]


def make_inputs(x, qkv_w, qkv_b, dw_w, dw_b, proj_w, proj_b, temperature,
                attn_w):
    """Host-side weight prep -> per-core input maps."""
    x = np.ascontiguousarray(np.asarray(x, np.float32)[:, :, :, 0])
    qkv_w = np.asarray(qkv_w, np.float32)
    qkv_b = np.asarray(qkv_b, np.float32)
    dw_w = np.asarray(dw_w, np.float32)
    dw_b = np.asarray(dw_b, np.float32)
    proj_w = np.asarray(proj_w, np.float32)
    proj_b = np.asarray(proj_b, np.float32)
    temperature = np.asarray(temperature, np.float32).reshape(HEADS)
    attn_w = np.asarray(attn_w, np.float32)

    dwk = dw_w[:, 0, :, 1]                       # [384, 3]

    wf8 = np.zeros((CDIM, WF8_COLS), np.float32)
    for t in range(3):
        wf8[:, t * 256:(t + 1) * 256] = (qkv_w[:256] * dwk[:256, t:t + 1]).T * SQK
    wf8[:, 1024] = 1.0
    wf8[:, 1152] = 1.0
    wf8 = wf8.astype(ml_dtypes.float8_e4m3fn)

    wb16 = np.zeros((CDIM, WB16_COLS), np.float32)
    for t in range(3):
        wb16[:, t * 128:(t + 1) * 128] = qkv_w[256:] * dwk[256:, t:t + 1]
    wb16[:, 384:512] = proj_w.T
    bv = qkv_b[256:] * dwk[256:].sum(1) + dw_b[256:]
    wb16[:, 512] = bv
    wb16[:, 513] = -qkv_b[256:] * dwk[256:, 0]
    wb16[:, 514] = -qkv_b[256:] * dwk[256:, 2]
    bqk = qkv_b[:256] * dwk[:256].sum(1) + dw_b[:256]
    wb16[0, 515:771] = bqk * SQ8
    wb16 = wb16.astype(ml_dtypes.bfloat16)
    b16f = wb16[0, 515:771].astype(np.float32)

    wf32 = np.zeros((CDIM, WF32_COLS), np.float32)
    wf32[:, 0:128] = np.eye(CDIM)
    wf32[:, 128] = proj_b
    wf32[:, 129] = np.repeat(np.log(np.maximum(temperature, 1e-30)), C)
    wf32[:, 130:134] = attn_w[None, :]
    wf32[:, 134] = 1.0
    wf32[127, 135:391] = -qkv_b[:256] * dwk[:256, 2] * SQ8

    wrow = np.zeros((1, WROW_COLS), np.float32)
    wrow[0, 0:256] = HALF_N * b16f
    wrow[0, 256:512] = -qkv_b[:256] * dwk[:256, 0] * SQ8
    wrow[0, 512:768] = -qkv_b[:256] * dwk[:256, 2] * SQ8
    wrow[0, 768:800] = 1.0

    const = dict(wf8=wf8, wb16=wb16, wf32=wf32, wrow=wrow)
    maps = []
    for i in range(NCORES):
        m = dict(const)
        m["x"] = np.ascontiguousarray(x[i * NSAMP:(i + 1) * NSAMP])
        maps.append(m)
    return maps


def kernel(**inputs):
    nc = _get_nc(debug=False)
    maps = make_inputs(**inputs)
    res = run_bass_kernel_spmd(nc, maps, list(range(NCORES)))
    outs = [res.results[i]["out"] for i in range(NCORES)]
    full = np.concatenate(outs, axis=0)           # [32, 128, 4096]
    return full[:, :, :, None].astype(np.float32)


# revision 15
# speedup vs baseline: 1.3858x; 1.1302x over previous
"""TRN2 Bass kernel for nn_DS_Block (sparse attention block).

Per sample (b=32, 4 per core over 8 cores), all-fused pipeline:
  x --fp8 DoubleRow 3-tap conv--> qkT [n,256] (fp8, scale 2^4)
  G = qkT^T qkT (fp8 DoubleRow gram) + rank-1 bf16 bias corrections
  rank-based 4-way top-k masked softmax on 32x32 head blocks
  P = proj @ A_blkdiag folded into the v-conv: out = sum_t (P Wv_t) x_shift
  (fp32r out conv; biases via PT^T @ [bv|evL|evR] matmul)
"""
import os
import sys
import json
from contextlib import ExitStack

sys.path.insert(0, "/opt/trn_rl_repo")

import numpy as np
import ml_dtypes
import bass_rust
import concourse.bass as bass
import concourse.mybir as mybir
import concourse.tile as tile
from concourse.bass_utils import run_bass_kernel_spmd

F32 = mybir.dt.float32
F32R = mybir.dt.float32r
F8 = mybir.dt.float8e4
BF16 = mybir.dt.bfloat16
AF = mybir.ActivationFunctionType
OP = mybir.AluOpType
DR = mybir.MatmulPerfMode.DoubleRow

B, CDIM, N = 32, 128, 4096
HEADS, C = 4, 32
NCORES = 8
NSAMP = B // NCORES
NPAIR = N // 256            # 16 chunk-pairs of 2x128
TOPKS = [C // 2, (2 * C) // 3, (3 * C) // 4, (4 * C) // 5]  # 16,21,24,25
SQK = 2.0 ** 5              # w8qk host scale == qk8 scale (moves are pure copies)
SQ8 = 2.0 ** 5              # qk8 fp8 scale
W2 = 512                    # per-chunk dual-plane block (planes at +0 / +256)
NCH = N // 128
HALF_N = (N // 2) * 1.0

# wf8 layout: [128, 4*256 + 130]: w8qk taps at j*256, ones-pair at 1024 & 1152
WF8_COLS = 1154
# wb16 layout: [128, 771]: w3vn (3*128), pwT (128), bv3 (3), b16row in row0 at 515:771
WB16_COLS = 771
# wf32 layout: [128, 391]: id128 (128), pb (1), lntemp (1), wS (4), onescol (1),
# eqkR-block (256: rows 96:127 zero, row 127 = eqkR)
WF32_COLS = 391
# wrow layout: [1, 800] f32: halfNb 0:256, eqkL 256:512, eqkR 512:768, ones32 768:800
WROW_COLS = 800


def _fix_sync_waits(bir: dict, max_waits: int = 1) -> dict:
    """This walrus build rejects >1 sem-wait per instruction; split the
    excess into standalone EventSemaphore waits on the same engine queue."""
    for f in bir.get("functions", []):
        for blk in f.get("blocks", []):
            out = []
            for inst in blk.get("instructions", []):
                si = inst.get("sync_info") or {}
                waits = si.get("on_wait") or []
                if len(waits) > max_waits:
                    extra, keep = waits[:-max_waits], waits[-max_waits:]
                    for j, w in enumerate(extra):
                        out.append({
                            "debug": inst.get("debug", 0),
                            "engine": inst["engine"],
                            "ins": [], "outs": [],
                            "name": f"{inst['name']}-xw{j}",
                            "opcode": "EventSemaphore",
                            "sync_info": {"on_update": [], "on_wait": [w]},
                        })
                    si["on_wait"] = keep
                    inst["sync_info"] = si
                out.append(inst)
            blk["instructions"] = out
    return bir


def _patch_nc(nc):
    orig = nc.to_json_bytes

    def fixed():
        return json.dumps(_fix_sync_waits(json.loads(orig()))).encode()

    nc.to_json_bytes = fixed
    return nc


def _ap(base, offset, dims):
    c = base.copy()
    c.ap = bass_rust.VecI64Pair(dims)
    c.offset = offset
    return c


def build_program():
    nc = bass.Bass("TRN2", target_bir_lowering=False, num_devices=NCORES)

    x_d = nc.dram_tensor("x", [NSAMP, CDIM, N], F32R, kind="ExternalInput")
    out_d = nc.dram_tensor("out", [NSAMP, CDIM, N], F32, kind="ExternalOutput")
    wf8_d = nc.dram_tensor("wf8", [CDIM, WF8_COLS], F8, kind="ExternalInput")
    wb16_d = nc.dram_tensor("wb16", [CDIM, WB16_COLS], BF16, kind="ExternalInput")
    wf32_d = nc.dram_tensor("wf32", [CDIM, WF32_COLS], F32, kind="ExternalInput")
    wrow_d = nc.dram_tensor("wrow", [1, WROW_COLS], F32, kind="ExternalInput")

    with tile.TileContext(nc) as tc, ExitStack() as es:
        wp = es.enter_context(tc.tile_pool(name="wp", bufs=1))
        xp = es.enter_context(tc.tile_pool(name="xp", bufs=3))
        x8p = es.enter_context(tc.tile_pool(name="x8p", bufs=2))
        qp = es.enter_context(tc.tile_pool(name="qp", bufs=3))
        bp = es.enter_context(tc.tile_pool(name="bp", bufs=2))
        op_ = es.enter_context(tc.tile_pool(name="op", bufs=2))
        ps_qk = es.enter_context(tc.tile_pool(name="ps_qk", bufs=2, space="PSUM"))
        ps_g = es.enter_context(tc.tile_pool(name="ps_g", bufs=2, space="PSUM"))
        ps_av = es.enter_context(tc.tile_pool(name="ps_av", bufs=2, space="PSUM"))
        ps_sm = es.enter_context(tc.tile_pool(name="ps_sm", bufs=2, space="PSUM"))

        # ---- constants ----
        wf8 = wp.tile([CDIM, WF8_COLS], F8)
        wb16 = wp.tile([CDIM, WB16_COLS], BF16)
        wf32 = wp.tile([CDIM, WF32_COLS], F32)
        wrow = wp.tile([1, WROW_COLS], F32)
        def emit_dma_w():
            nc.sync.dma_start(wf8[:], wf8_d[:])
            nc.sync.dma_start(wb16[:], wb16_d[:])
            nc.sync.dma_start(wf32[:], wf32_d[:])
            nc.sync.dma_start(wrow[:], wrow_d[:])
        id128 = wf32[:, 0:128]
        pb = wf32[:, 128:129]
        lntemp = wf32[:, 129:130]
        wS = wf32[:, 130:134]
        onescol = wf32[:, 134:135]
        eqkRblk = wf32[96:128, 135:391]
        w3vn = lambda t: wb16[:, t * 128:(t + 1) * 128]
        pwTb = wb16[:, 384:512]
        bv3 = wb16[:, 512:515]
        b16row = wb16[0:1, 515:771]       # scaled bias row (2^4), bf16
        halfNb = wrow[0:1, 0:256]
        eqkL = wrow[0:1, 256:512]
        eqkR = wrow[0:1, 512:768]
        ones32 = wrow[0:1, 768:800]

        PW = list(wf8[:].ap[0])           # wf8 partition AP pair

        # per-sample emission state
        xts, x8s, sms, Gs, Bq = {}, {}, {}, {}, {}
        tiles = {}

        xdma_done = {}

        def emit_dma_x(s, upto=2):
            if s not in xts:
                xt = xp.tile([CDIM, N + 8], F32R, tag="x", name=f"x{s}")
                xts[s] = xt
                xdma_done[s] = 0
                if s < 3:
                    xf = xt[:].bitcast(F32)
                    nc.vector.memset(xf[:, 0:1], 0.0)
                    nc.vector.memset(xf[:, N + 1:N + 8], 0.0)
            xt = xts[s]
            while xdma_done[s] < upto:
                h = xdma_done[s]
                nc.sync.dma_start(xt[:, 1 + h * 2048:1 + (h + 1) * 2048],
                                  x_d[s, :, h * 2048:(h + 1) * 2048])
                xdma_done[s] = h + 1

        def convert_pieces(s):
            # fp8 per-chunk dual-plane blocks: chunk ch at col ch*W2:
            #   [0:132]   = xpad[ch*128 + 0 .. 131]  (tap base, shift 0)
            #   [256:388] = xpad[ch*128 + 1 .. 132]  (shift +1)
            # split into 4-chunk pieces across ACT / DVE / Pool
            xt = xts[s]
            x8 = x8p.tile([CDIM, NCH * W2], F8, tag="x8", name=f"x8_{s}")
            x8s[s] = x8
            xb, o8 = xt[:], x8[:]
            PXT, PX8 = list(xb.ap[0]), list(o8.ap[0])
            ops = []
            for p in range(8):
                def f(p=p):
                    dst = _ap(o8, p * 4 * W2, [PX8, [W2, 4], [1, 132]])
                    sc = _ap(xb, p * 4 * 128, [PXT, [128, 4], [1, 132]])
                    dst1 = _ap(o8, p * 4 * W2 + 256, [PX8, [W2, 4], [1, 132]])
                    sc1 = _ap(xb, p * 4 * 128 + 1, [PXT, [128, 4], [1, 132]])
                    if p % 2 == 0:
                        nc.scalar.activation(dst, sc, AF.Identity)
                        nc.gpsimd.tensor_copy(dst1, sc1)
                    else:
                        nc.gpsimd.tensor_copy(dst, sc)
                        nc.vector.tensor_copy(dst1, sc1)
                ops.append(f)
            return ops

        def emit_A(s, interleave=None):
            # qkT + gram + srow for sample s; pops B(s-1) closures as it goes
            x8 = x8s[s]
            G = ps_g.tile([CDIM, 512], F32, tag="g", name=f"G{s}")
            Gs[s] = G
            sm = ps_sm.tile([CDIM, 512], F32, tag="sm", name=f"sm{s}")
            sms[s] = sm
            x8b = x8[:]
            PX = list(x8b.ap[0])
            for cp in range(NPAIR):
                qkps = ps_qk.tile([CDIM, 512], F32, tag="qk", name=f"qkps{s}_{cp}")
                for half in range(2):
                    ch = cp * 2 + half
                    o = half * 256
                    # taps (0,1): lhsT = chunk dual-plane block, j stride 132
                    nc.tensor.matmul(
                        qkps[:, o:o + 256],
                        _ap(x8b, ch * W2, [PX, [256, 2], [1, 128]]),
                        _ap(wf8[:], 0, [PW, [256, 2], [1, 256]]),
                        perf_mode=DR, start=True, stop=False)
                    # taps (2,3): block offsets +2; tap3 weights are zero
                    nc.tensor.matmul(
                        qkps[:, o:o + 256],
                        _ap(x8b, ch * W2 + 2, [PX, [256, 2], [1, 128]]),
                        _ap(wf8[:], 512, [PW, [256, 2], [1, 256]]),
                        perf_mode=DR, start=False, stop=True)
                qk8 = qp.tile([CDIM, 512], F8, tag="qk8", name=f"qk8_{s}_{cp}")
                if cp % 2 == 0:
                    nc.vector.tensor_copy(qk8[:], qkps[:])
                else:
                    nc.scalar.copy(qk8[:], qkps[:])
                if cp == 0:   # n=0 edge fix (bias tap overcount)
                    nc.vector.tensor_tensor(qk8[0:1, 0:256], qk8[0:1, 0:256],
                                            eqkL[:], OP.add)
                if cp == NPAIR - 1:  # n=4095 edge fix (rows 96:127 add zero)
                    nc.vector.tensor_tensor(qk8[96:128, 256:512],
                                            qk8[96:128, 256:512], eqkRblk,
                                            OP.add)
                qb = qk8[:]
                PQ = list(qb.ap[0])
                st, sp_ = (cp == 0), False
                # gram A: [QtQ | QtK] <- q^T (q|k), both chunks via DoubleRow
                nc.tensor.matmul(G[:, 0:256],
                                 _ap(qb, 0, [PQ, [256, 2], [1, 128]]),
                                 _ap(qb, 0, [PQ, [256, 2], [1, 256]]),
                                 perf_mode=DR, start=st, stop=sp_)
                # gram B: KtK at cols 256:384
                nc.tensor.matmul(G[:, 256:384],
                                 _ap(qb, 128, [PQ, [256, 2], [1, 128]]),
                                 _ap(qb, 128, [PQ, [256, 2], [1, 128]]),
                                 perf_mode=DR, start=st, stop=sp_)
                # srow: column sums via ones-pair
                nc.tensor.matmul(sm[0:1, 0:256],
                                 _ap(wf8[:], 1024, [PW, [128, 2], [1, 1]]),
                                 _ap(qb, 0, [PQ, [256, 2], [1, 256]]),
                                 perf_mode=DR, start=st, stop=(cp == NPAIR - 1))
                if interleave:
                    interleave(cp)
            # bias as rank-1: G += b (x) u + u (x) b with u = srow + (N/2) b
            u16 = bp.tile([1, 256], BF16, tag="u16", name=f"u16_{s}")
            nc.vector.scalar_tensor_tensor(u16[:], sm[0:1, 0:256], 1.0,
                                           halfNb[:], OP.mult, OP.add)
            ub = u16[:]
            nc.tensor.matmul(G[:, 0:256], b16row[0:1, 0:128], ub,
                             start=False, stop=False)
            nc.tensor.matmul(G[:, 0:256], u16[0:1, 0:128], b16row[:],
                             start=False, stop=True)
            nc.tensor.matmul(G[:, 256:384], b16row[0:1, 128:256],
                             u16[0:1, 128:256], start=False, stop=False)
            nc.tensor.matmul(G[:, 256:384], u16[0:1, 128:256],
                             b16row[0:1, 128:256], start=False, stop=True)

        def build_B(s):
            # attention phase closures (interleaved into A(s+1) emission)
            G, sm = Gs[s], sms[s]
            t = {}
            t["scrQ"] = bp.tile([CDIM, 128], F32, tag="scrQ", name=f"scrQ{s}")
            t["scrK"] = bp.tile([CDIM, 128], F32, tag="scrK", name=f"scrK{s}")
            t["qsq"] = bp.tile([CDIM, 1], F32, tag="qsq", name=f"qsq{s}")
            t["rq"] = bp.tile([CDIM, 1], F32, tag="rq", name=f"rq{s}")
            t["rkrow"] = bp.tile([1, 128], F32, tag="rkrow", name=f"rkrow{s}")
            t["attn_raw"] = bp.tile([CDIM, 32], F32, tag="attn_raw", name=f"ar{s}")
            t["attn_s"] = bp.tile([CDIM, 32], F32, tag="attn_s", name=f"as{s}")
            t["nmx"] = bp.tile([CDIM, 1], F32, tag="nmx", name=f"nmx{s}")
            t["e"] = bp.tile([CDIM, 32], F32, tag="e", name=f"e{s}")
            t["cmp"] = bp.tile([CDIM, C, C], F32, tag="cmp", name=f"cmp{s}")
            t["rank"] = bp.tile([CDIM, 32], F32, tag="rank", name=f"rank{s}")
            t["S"] = bp.tile([CDIM, 4], F32, tag="S", name=f"S{s}")
            t["R"] = bp.tile([CDIM, 4], F32, tag="R", name=f"R{s}")
            t["A"] = bp.tile([CDIM, 32], F32, tag="A", name=f"A{s}")
            t["me"] = [bp.tile([CDIM, 32], F32, tag=f"me{i}", name=f"me{i}_{s}")
                       for i in range(4)]
            tiles[s] = t
            ops = []
            ops.append(lambda: nc.vector.tensor_tensor(
                t["scrK"][:], G[:, 256:384], id128, OP.mult))
            ops.append(lambda: nc.tensor.matmul(
                sm[0:1, 256:384], onescol, t["scrK"][:]))
            ops.append(lambda: (
                nc.scalar.activation(t["rkrow"][:], sm[0:1, 256:384], AF.Ln),
                nc.scalar.activation(t["rkrow"][:], t["rkrow"][:], AF.Exp,
                                     scale=-0.5)))
            ops.append(lambda: nc.vector.tensor_tensor(
                t["scrQ"][:], G[:, 0:128], id128, OP.mult))
            ops.append(lambda: nc.vector.tensor_reduce(
                t["qsq"][:], t["scrQ"][:], mybir.AxisListType.X, OP.add))
            ops.append(lambda: (
                nc.scalar.activation(t["rq"][:], t["qsq"][:], AF.Ln),
                nc.scalar.activation(t["rq"][:], t["rq"][:], AF.Exp,
                                     bias=lntemp, scale=-0.5)))
            def rk_mms():
                for h in range(HEADS):
                    nc.tensor.matmul(sm[32 * h:32 * (h + 1), 384:416],
                                     ones32[:], t["rkrow"][0:1, 32 * h:32 * h + 32],
                                     tile_position=(0, 32 * h))
            ops.append(rk_mms)
            def ar_copies():
                for h in range(HEADS):
                    nc.scalar.copy(t["attn_raw"][32 * h:32 * (h + 1), :],
                                   G[32 * h:32 * (h + 1),
                                     128 + 32 * h:128 + 32 * (h + 1)])
            ops.append(ar_copies)
            ops.append(lambda: nc.vector.scalar_tensor_tensor(
                t["attn_s"][:], t["attn_raw"][:], t["rq"][:],
                sm[:, 384:416], OP.mult, OP.mult))
            ops.append(lambda: nc.vector.tensor_reduce(
                t["nmx"][:], t["attn_s"][:], mybir.AxisListType.X, OP.max,
                negate=True))
            ops.append(lambda: nc.scalar.activation(
                t["e"][:], t["attn_s"][:], AF.Exp, bias=t["nmx"][:]))
            ops.append(lambda: nc.vector.tensor_tensor(
                t["cmp"][:], t["attn_s"][:, :, None].to_broadcast((CDIM, C, C)),
                t["attn_s"][:, None, :].to_broadcast((CDIM, C, C)), OP.is_lt))
            ops.append(lambda: nc.vector.tensor_reduce(
                t["rank"][:], t["cmp"][:], mybir.AxisListType.X, OP.add))
            def masked(i):
                return lambda: nc.vector.scalar_tensor_tensor(
                    t["me"][i][:], t["rank"][:], float(TOPKS[i]), t["e"][:],
                    OP.is_lt, OP.mult, accum_out=t["S"][:, i:i + 1])
            for i in range(4):
                ops.append(masked(i))
            def combine():
                nc.vector.reciprocal(t["R"][:], t["S"][:])
                nc.vector.tensor_tensor(t["R"][:], t["R"][:], wS, OP.mult)
                nc.vector.tensor_scalar_mul(t["A"][:], t["me"][0][:],
                                            t["R"][:, 0:1])
                for i in range(1, 4):
                    nc.vector.scalar_tensor_tensor(
                        t["A"][:], t["me"][i][:], t["R"][:, i:i + 1], t["A"][:],
                        OP.mult, OP.add)
            ops.append(combine)
            Bq[s] = ops

        def emit_C(s):
            # P-fold + out conv + store for sample s (after B(s) complete)
            t, sm, xt = tiles[s], sms[s], xts[s]
            BD = bp.tile([CDIM, CDIM], BF16, tag="BD", name=f"BD{s}")
            if s < 2:
                nc.vector.memset(BD[:], 0.0)
            for h in range(HEADS):
                nc.scalar.copy(BD[32 * h:32 * (h + 1), 32 * h:32 * (h + 1)],
                               t["A"][32 * h:32 * (h + 1), :])
            nc.tensor.matmul(sm[:, 384:512], BD[:], pwTb)
            PTb = bp.tile([CDIM, CDIM], BF16, tag="PTb", name=f"PTb{s}")
            nc.scalar.copy(PTb[:], sm[:, 384:512])
            MTps = ps_av.tile([CDIM, 384], F32, tag="av", name=f"MTps{s}")
            for tp in range(3):
                nc.tensor.matmul(MTps[:, tp * 128:(tp + 1) * 128],
                                 w3vn(tp), PTb[:])
            MTf = bp.tile([CDIM, 384], F32R, tag="MTf", name=f"MTf{s}")
            nc.scalar.copy(MTf[:], MTps[:])
            nc.tensor.matmul(sm[:, 416:419], PTb[:], bv3)
            biasv = bp.tile([CDIM, 1], F32, tag="biasv", name=f"biasv{s}")
            nc.vector.tensor_tensor(biasv[:], pb, sm[:, 416:417], OP.add)
            ot = op_.tile([CDIM, N], F32, tag="out", name=f"ot{s}")
            for q in range(8):
                avps = ps_av.tile([CDIM, 512], F32, tag="av", name=f"av{s}_{q}")
                for tp in range(3):
                    nc.tensor.matmul(avps[:],
                                     MTf[:, tp * 128:(tp + 1) * 128],
                                     xt[:, q * 512 + tp:q * 512 + tp + 512],
                                     start=(tp == 0), stop=(tp == 2))
                if q % 2 == 0:
                    nc.scalar.activation(ot[:, q * 512:(q + 1) * 512], avps[:],
                                         AF.Identity, bias=biasv[:])
                else:
                    nc.vector.tensor_scalar_add(ot[:, q * 512:(q + 1) * 512],
                                                avps[:], biasv[:])
                if q == 0:
                    nc.vector.tensor_tensor(ot[:, 0:1], ot[:, 0:1],
                                            sm[:, 417:418], OP.add)
                if q == 7:
                    nc.vector.tensor_tensor(ot[:, N - 1:N], ot[:, N - 1:N],
                                            sm[:, 418:419], OP.add)
                if q == 3:
                    nc.sync.dma_start(out_d[s, :, 0:2048], ot[:, 0:2048])
            nc.sync.dma_start(out_d[s, :, 2048:N], ot[:, 2048:N])

        # ---- pipeline ----
        emit_dma_x(0, upto=1)
        emit_dma_w()
        emit_dma_x(0, upto=2)
        for f in convert_pieces(0):
            f()
        emit_dma_x(1)
        for s in range(NSAMP):
            work = []
            bops = Bq.pop(s - 1, [])
            cops = convert_pieces(s + 1) if s + 1 < NSAMP else []
            n = max(len(bops), len(cops))
            for i in range(n):
                if i < len(cops):
                    work.append(cops[i])
                if i < len(bops):
                    work.append(bops[i])
            def inter(cp, _p=work):
                budget = 3 if cp < 8 else 2
                for _ in range(budget):
                    if _p:
                        _p.pop(0)()
            emit_A(s, interleave=inter if work else None)
            for f in work:
                f()
            build_B(s)
            if s >= 1:
                emit_C(s - 1)
            if s + 2 < NSAMP:
                emit_dma_x(s + 2)
        for f in Bq[NSAMP - 1]:
            f()
        emit_C(NSAMP - 1)

    _patch_nc(nc)
    return nc


_NC_CACHE = {}


def _get_nc(debug=False):
    key = bool(debug)
    if key not in _NC_CACHE:
        _NC_CACHE[key] = build_program()
    return _NC_CACHE[key]


def make_inputs(x, qkv_w, qkv_b, dw_w, dw_b, proj_w, proj_b, temperature,
                attn_w):
    """Host-side weight prep -> per-core input maps."""
    x = np.ascontiguousarray(np.asarray(x, np.float32)[:, :, :, 0])
    qkv_w = np.asarray(qkv_w, np.float32)
    qkv_b = np.asarray(qkv_b, np.float32)
    dw_w = np.asarray(dw_w, np.float32)
    dw_b = np.asarray(dw_b, np.float32)
    proj_w = np.asarray(proj_w, np.float32)
    proj_b = np.asarray(proj_b, np.float32)
    temperature = np.asarray(temperature, np.float32).reshape(HEADS)
    attn_w = np.asarray(attn_w, np.float32)

    dwk = dw_w[:, 0, :, 1]                       # [384, 3]

    wf8 = np.zeros((CDIM, WF8_COLS), np.float32)
    for t in range(3):
        wf8[:, t * 256:(t + 1) * 256] = (qkv_w[:256] * dwk[:256, t:t + 1]).T * SQK
    wf8[:, 1024] = 1.0
    wf8[:, 1152] = 1.0
    wf8 = wf8.astype(ml_dtypes.float8_e4m3fn)

    wb16 = np.zeros((CDIM, WB16_COLS), np.float32)
    for t in range(3):
        wb16[:, t * 128:(t + 1) * 128] = qkv_w[256:] * dwk[256:, t:t + 1]
    wb16[:, 384:512] = proj_w.T
    bv = qkv_b[256:] * dwk[256:].sum(1) + dw_b[256:]
    wb16[:, 512] = bv
    wb16[:, 513] = -qkv_b[256:] * dwk[256:, 0]
    wb16[:, 514] = -qkv_b[256:] * dwk[256:, 2]
    bqk = qkv_b[:256] * dwk[:256].sum(1) + dw_b[:256]
    wb16[0, 515:771] = bqk * SQ8
    wb16 = wb16.astype(ml_dtypes.bfloat16)
    b16f = wb16[0, 515:771].astype(np.float32)

    wf32 = np.zeros((CDIM, WF32_COLS), np.float32)
    wf32[:, 0:128] = np.eye(CDIM)
    wf32[:, 128] = proj_b
    wf32[:, 129] = np.repeat(np.log(np.maximum(temperature, 1e-30)), C)
    wf32[:, 130:134] = attn_w[None, :]
    wf32[:, 134] = 1.0
    wf32[127, 135:391] = -qkv_b[:256] * dwk[:256, 2] * SQ8

    wrow = np.zeros((1, WROW_COLS), np.float32)
    wrow[0, 0:256] = HALF_N * b16f
    wrow[0, 256:512] = -qkv_b[:256] * dwk[:256, 0] * SQ8
    wrow[0, 512:768] = -qkv_b[:256] * dwk[:256, 2] * SQ8
    wrow[0, 768:800] = 1.0

    const = dict(wf8=wf8, wb16=wb16, wf32=wf32, wrow=wrow)
    maps = []
    for i in range(NCORES):
        m = dict(const)
        m["x"] = np.ascontiguousarray(x[i * NSAMP:(i + 1) * NSAMP])
        maps.append(m)
    return maps


def kernel(**inputs):
    nc = _get_nc(debug=False)
    maps = make_inputs(**inputs)
    res = run_bass_kernel_spmd(nc, maps, list(range(NCORES)))
    outs = [res.results[i]["out"] for i in range(NCORES)]
    full = np.concatenate(outs, axis=0)           # [32, 128, 4096]
    return full[:, :, :, None].astype(np.float32)
